# revision 21
# baseline (speedup 1.0000x reference)
"""Trainium2 Bass kernel for CustomSimplexMappingAttention (causal sparsemax attention).

Problem: y = (sparsemax(causal(Q K^T / sqrt(hd))) V) W_o^T with
B=2, L=2048, D=1024, H=16, hd=64, all fp32.

Sharding: batch*heads across 8 cores. Core c handles batch b = c//4 and the
4 heads [4*(c%4), 4*(c%4)+4). Each core computes a partial y for its batch
(row-parallel W_o); host sums the 4 partials per batch (the "all-reduce").

v2 design (per core; 2 head-groups g, each packing heads h0/h1 on partition
halves 0-63 / 64-127):
  Phase 1: q^T/k^T [128, L] per group (PE, K=128 contraction over 8 d-chunks)
    and v in NATURAL layout vn[key, head*64] (no PE transposes needed).
  Stage A (per group): natural scores z[q, k] for both heads CONCURRENTLY via
    PE row-tiling (tile_position (0,0)/(64,0), K=64 each). Causal diag masking
    is done ON PE by accumulating ident.T @ triu_mask (bf16, N=128 @ 1cyc/row).
    DVE max8 extracts top-8 of each 512-wide PSUM tile -> 32 candidates/row.
  Solver: top-16 refinement (max8+match_replace+max8 -> SORTED desc c16),
    then closed-form sparsemax threshold: tau = max_r (cssv_r - 1)/r
    (exact for sorted input). 3 DVE ops: segmented prefix-scan
    (tensor_tensor_scan with a 0/1 segment mask), fused (cssv-1)*(-1/r)
    (scalar_tensor_tensor), reduce-min -> -tau. -tau rows are transposed and
    DMA'd into single-partition rows for the Stage B rank-1 subtraction.
  Stage B (per group): scores recomputed TRANSPOSED (z^T[k, q]) with causal
    narrowing (skip fully-masked column ranges), both heads row-tiled into one
    2-bank [128, 1024] PSUM tile; -tau added via a K=2 bf16 matmul (ones x
    [tau_hi; tau_lo] rows -- the bf16 pair recovers fp32 precision); diag
    masking again via ident.T @ mask accumulation (bf16). Full-width blocks
    relu-evict attn^T (bf16) in ONE paired ACT call (split ACT/DVE on the
    last group where DVE is otherwise idle); PV accumulates v-stationary bf16
    matmuls with the two heads COL-TILED (tile_position (0,0)/(0,64)) into a
    [128, 512] PSUM tile = out^T pair. The pv accumulation group is opened and
    closed by full-width zero rank-1 matmuls so the whole 128-partition bank
    is one unambiguous accumulation group.
  Phase 3 (pipelined per q-chunk of the last group): y tiles =
    sum_g opT[g].T @ woT[g] (K=128), evicted alternately on ACT/DVE.

niter > 1 wraps the whole body in a hardware For_i loop (same work each
iteration, including input DMAs) so per-iteration device time can be measured
without the multi-ms per-call host dispatch overhead of this environment.
"""

import numpy as np

B, L, D, H, HD = 2, 2048, 1024, 16, 64
NEG = -1e9
N_CORES = 8
HEADS_PER_CORE = 4
NCAND = 16
VERSION = "v2"


def build_program(Lk=L, niter=1):
    """Build the Bass program for one core (SPMD: all cores run this)."""
    import concourse.bacc as bacc
    import concourse.bass as bass
    import concourse.mybir as mybir
    import concourse.tile as tile

    fp32 = mybir.dt.float32
    fp32r = mybir.dt.float32r
    bf16 = mybir.dt.bfloat16
    ALU = mybir.AluOpType
    ACTF = mybir.ActivationFunctionType

    n_lt = Lk // 128     # 128-row tiles
    n_qc = Lk // 512     # 512-wide query chunks

    nc = bacc.Bacc("TRN2", target_bir_lowering=False, debug=False)

    # ---- DRAM I/O ----
    xT_d = nc.dram_tensor("xT", [D, Lk], fp32r, kind="ExternalInput").ap()
    wqT_d = nc.dram_tensor("wqT", [D, 256], fp32r, kind="ExternalInput").ap()
    wkT_d = nc.dram_tensor("wkT", [D, 256], fp32r, kind="ExternalInput").ap()
    wvT_d = nc.dram_tensor("wvT", [D, 256], fp32r, kind="ExternalInput").ap()
    woT_d = nc.dram_tensor("woT", [256, D], fp32r, kind="ExternalInput").ap()
    identF_d = nc.dram_tensor("identF", [128, 128], fp32, kind="ExternalInput").ap()
    identB_d = nc.dram_tensor("identB", [128, 128], bf16, kind="ExternalInput").ap()
    triUA_d = nc.dram_tensor("triUA", [128, 128], bf16, kind="ExternalInput").ap()
    triLB_d = nc.dram_tensor("triLB", [128, 128], bf16, kind="ExternalInput").ap()
    fullB_d = nc.dram_tensor("fullB", [128, 128], bf16, kind="ExternalInput").ap()
    ones_d = nc.dram_tensor("ones", [128, 128], bf16, kind="ExternalInput").ap()
    nrinv_d = nc.dram_tensor("nrinv", [128, NCAND], fp32, kind="ExternalInput").ap()
    y_d = nc.dram_tensor("y", [Lk, D], fp32, kind="ExternalOutput").ap()

    def bc_mid(ap, n):
        # broadcast a [P, M] AP along a new middle (stride-0) dim of size n
        return bass.AP(tensor=ap.tensor, offset=ap.offset,
                       ap=[list(ap.ap[0]), [0, n]] + [list(d) for d in ap.ap[1:]])

    def body(tc):
        with tc.tile_pool(name="persist", bufs=1) as persist:
            qT = [persist.tile([128, Lk], fp32r, tag=f"qT{g}", name=f"qT{g}") for g in range(2)]
            kT = [persist.tile([128, Lk], fp32r, tag=f"kT{g}", name=f"kT{g}") for g in range(2)]
            vn = persist.tile([128, n_lt, 256], bf16, tag="vn", name="vn")
            opT = [persist.tile([128, Lk], fp32r, tag=f"opT{g}", name=f"opT{g}") for g in range(2)]
            ntau_row = persist.tile([128, Lk], bf16, tag="ntau_row")

            identF = persist.tile([128, 128], fp32, tag="identF")
            identB = persist.tile([128, 128], bf16, tag="identB")
            triUA = persist.tile([128, 128], bf16, tag="triUA")
            triLB = persist.tile([128, 128], bf16, tag="triLB")
            fullB = persist.tile([128, 128], bf16, tag="fullB")
            onesB = persist.tile([128, 128], bf16, tag="ones")
            nrinv = persist.tile([128, NCAND], fp32, tag="nrinv")
            zrow = persist.tile([1, 128], bf16, tag="zrow")
            segmask = persist.tile([128, n_lt, NCAND], fp32, tag="segmask")

            nc.sync.dma_start(out=identF, in_=identF_d)
            nc.sync.dma_start(out=identB, in_=identB_d)
            nc.sync.dma_start(out=triUA, in_=triUA_d)
            nc.sync.dma_start(out=triLB, in_=triLB_d)
            nc.sync.dma_start(out=fullB, in_=fullB_d)
            nc.sync.dma_start(out=onesB, in_=ones_d)
            nc.sync.dma_start(out=nrinv, in_=nrinv_d)
            nc.vector.memset(zrow, 0.0)
            # segment mask for the batched per-row-tile prefix scan:
            # 0 at each 16-candidate segment start, 1 elsewhere
            nc.vector.memset(segmask, 1.0)
            nc.vector.memset(segmask[:, :, 0:1], 0.0)

            with tc.tile_pool(name="xw", bufs=1) as xw:
                xT = xw.tile([128, 8, Lk], fp32r, tag="xT")
                wq = xw.tile([128, 8, 256], fp32r, tag="wq")
                wk = xw.tile([128, 8, 256], fp32r, tag="wk")
                wv = xw.tile([128, 8, 256], fp32r, tag="wv")
                woT2 = xw.tile([128, 2, D], fp32r, tag="woT2")
                # first projection matmuls need wq[0] + the head of xT[0]:
                # issue those first, then stream the rest
                nc.sync.dma_start(out=wq[:, 0, :], in_=wqT_d[0:128, :])
                for cchunk in range(4):
                    nc.sync.dma_start(
                        out=xT[:, 0, 512 * cchunk:512 * (cchunk + 1)],
                        in_=xT_d[0:128, 512 * cchunk:512 * (cchunk + 1)])
                for dc in range(1, 8):
                    nc.sync.dma_start(out=xT[:, dc, :], in_=xT_d[128 * dc:128 * (dc + 1), :])
                    nc.sync.dma_start(out=wq[:, dc, :], in_=wqT_d[128 * dc:128 * (dc + 1), :])
                for dc in range(8):
                    nc.sync.dma_start(out=wk[:, dc, :], in_=wkT_d[128 * dc:128 * (dc + 1), :])
                    nc.sync.dma_start(out=wv[:, dc, :], in_=wvT_d[128 * dc:128 * (dc + 1), :])
                for p in range(2):
                    nc.sync.dma_start(out=woT2[:, p, :], in_=woT_d[128 * p:128 * (p + 1), :])

                # ---------- Pipelined phases ----------
                # Emission order = scheduler priority. Stage A of group 0 is
                # emitted right after the g0 q/k projections so the DVE
                # candidate scan (the serial backbone) starts ~30us earlier;
                # the g1/v projections then fill the PE under it. All PSUM
                # pools coexist: proj/v/z/phase3 share the "z" tag (3 banks)
                # + st (2x2 banks) + pv (1 bank) = 8 banks.
                with tc.tile_pool(name="zps", bufs=3, space="PSUM") as zps, \
                     tc.tile_pool(name="stps", bufs=2, space="PSUM") as stps, \
                     tc.tile_pool(name="pvps", bufs=1, space="PSUM") as pvps, \
                     tc.tile_pool(name="cands", bufs=2) as cands, \
                     tc.tile_pool(name="solver", bufs=2) as solver, \
                     tc.tile_pool(name="attn", bufs=4) as attnp, \
                     tc.tile_pool(name="small", bufs=4) as small, \
                     tc.tile_pool(name="yout", bufs=2) as yout:

                    def proj_qk(g):
                        for dst, w in ((qT[g], wq), (kT[g], wk)):
                            # dc-outer over qc pairs: weight load serves 2 mms
                            for qp in range(n_qc // 2):
                                pss = [zps.tile([128, 512], fp32, tag="z",
                                                name=f"proj{j}")
                                       for j in range(2)]
                                for dc in range(8):
                                    for j in range(2):
                                        qc = 2 * qp + j
                                        nc.tensor.matmul(
                                            pss[j],
                                            lhsT=w[:, dc, 128 * g:128 * (g + 1)],
                                            rhs=xT[:, dc, 512 * qc:512 * (qc + 1)],
                                            start=(dc == 0), stop=(dc == 7),
                                        )
                                for j in range(2):
                                    qc = 2 * qp + j
                                    nc.scalar.copy(
                                        dst[:, 512 * qc:512 * (qc + 1)], pss[j])

                    def proj_v():
                        # v natural: vn[key, head*64] = x @ Wv^T
                        for j in range(n_lt):
                            psv = zps.tile([128, 256], fp32, tag="z", name="psv")
                            for dc in range(8):
                                nc.tensor.matmul(
                                    psv,
                                    lhsT=xT[:, dc, 128 * j:128 * (j + 1)],
                                    rhs=wv[:, dc, :],
                                    start=(dc == 0), stop=(dc == 7),
                                )
                            nc.scalar.copy(vn[:, j, :], psv)

                    def stage_a(g):
                        # natural scores -> per-512-chunk top8 -> sorted top16
                        cand = [cands.tile([128, n_lt, 32], fp32, tag=f"cand{h}",
                                           name=f"cand{h}")
                                for h in range(2)]
                        c16s = [solver.tile([128, n_lt, NCAND], fp32, tag=f"c16_{h}",
                                            name=f"c16_{h}")
                                for h in range(2)]
                        scrs = [solver.tile([128, 32], fp32, tag=f"scr{h}",
                                            name=f"scr{h}")
                                for h in range(2)]
                        nc.vector.memset(cand[0], NEG)
                        nc.vector.memset(cand[1], NEG)
                        for i in range(n_lt):
                            W = 128 * (i + 1)
                            for wc0 in range(0, W, 512):
                                wcw = min(512, W - wc0)
                                diag = (wc0 + wcw == W)
                                cslot = wc0 // 512
                                for h in range(2):
                                    hs = slice(64 * h, 64 * (h + 1))
                                    zp = zps.tile([128, 512], fp32, tag="z")
                                    nc.tensor.matmul(
                                        zp[:, :wcw],
                                        lhsT=qT[g][hs, 128 * i:128 * (i + 1)],
                                        rhs=kT[g][hs, wc0:wc0 + wcw],
                                        start=True, stop=not diag,
                                        tile_position=(64 * h, 0),
                                    )
                                    if diag:
                                        nc.tensor.matmul(
                                            zp[:, wcw - 128:wcw],
                                            lhsT=identB, rhs=triUA,
                                            start=False, stop=True,
                                        )
                                    nc.vector.max(
                                        out=cand[h][:, i, 8 * cslot:8 * cslot + 8],
                                        in_=zp[:, :wcw])
                            # top-16 refinement for row-tile i (sorted desc)
                            for h in range(2):
                                nc.vector.max(out=c16s[h][:, i, 0:8],
                                              in_=cand[h][:, i, :])
                                nc.vector.match_replace(
                                    out=scrs[h], in_to_replace=c16s[h][:, i, 0:8],
                                    in_values=cand[h][:, i, :], imm_value=NEG)
                                nc.vector.max(out=c16s[h][:, i, 8:16], in_=scrs[h])
                        return c16s

                    def solver_tau(g, c16s):
                        # closed-form sparsemax tau from sorted candidates
                        for h in range(2):
                            c16 = c16s[h]
                            cssv = solver.tile([128, n_lt, NCAND], fp32, tag="cssv")
                            flat = "p a b -> p (a b)"
                            nc.vector.tensor_tensor_scan(
                                cssv.rearrange(flat), segmask.rearrange(flat),
                                c16.rearrange(flat), 0.0, ALU.mult, ALU.add)
                            nc.vector.scalar_tensor_tensor(
                                out=cssv, in0=cssv, scalar=-1.0,
                                in1=bc_mid(nrinv, n_lt), op0=ALU.add, op1=ALU.mult)
                            ntau = solver.tile([128, n_lt], fp32, tag="ntau")
                            nc.vector.tensor_reduce(
                                out=ntau, in_=cssv, axis=mybir.AxisListType.X,
                                op=ALU.min)
                            # -tau as bf16 hi/lo row pair at partitions p_h,
                            # p_h+1 (hi + residual recovers fp32 precision);
                            # one packed [32,128] tile -> ONE row-pair DMA
                            p_h = 64 * h + 32 * g
                            tps = stps.tile([128, 512], fp32, tag="st")
                            nc.tensor.transpose(tps[:n_lt, :128], ntau, identF)
                            hiT = small.tile([n_lt, 128], bf16, tag="hiT")
                            hiF = small.tile([n_lt, 128], fp32, tag="hiF")
                            loT = small.tile([n_lt, 128], bf16, tag="loT")
                            nc.vector.tensor_copy(out=hiT, in_=tps[:n_lt, :128])
                            nc.vector.tensor_copy(out=hiF, in_=hiT)
                            nc.vector.tensor_tensor(
                                out=loT, in0=tps[:n_lt, :128], in1=hiF,
                                op=ALU.subtract)
                            nc.sync.dma_start(
                                out=ntau_row[p_h:p_h + 1, :].rearrange(
                                    "p (a b) -> p a b", b=128),
                                in_=hiT)
                            nc.sync.dma_start(
                                out=ntau_row[p_h + 1:p_h + 2, :].rearrange(
                                    "p (a b) -> p a b", b=128),
                                in_=loT)

                    def stage_b(g):
                        # transposed scores + tau + masks -> relu -> PV
                        for qc in range(n_qc):
                            pv = pvps.tile([128, 512], fp32, tag="pv")
                            kt_hi = 4 * qc + 3
                            nc.tensor.matmul(
                                pv, lhsT=zrow, rhs=ntau_row[0:1, 0:512],
                                start=True, stop=False,
                            )
                            for kt in range(kt_hi + 1):
                                d = kt - 4 * qc
                                c0 = 0 if d < 0 else min(128 * d, 256)
                                # both heads' scores side by side in one
                                # 2-bank PSUM tile (independent zero regions)
                                st2 = stps.tile([128, 1024], fp32, tag="st")
                                a2 = attnp.tile([128, 1024], bf16, tag="at")
                                for h in range(2):
                                    hs = slice(64 * h, 64 * (h + 1))
                                    p_h = 64 * h + 32 * g
                                    hb = 512 * h
                                    st = st2[:, hb:hb + 512]
                                    nc.tensor.matmul(
                                        st[:, c0:],
                                        lhsT=kT[g][hs, 128 * kt:128 * (kt + 1)],
                                        rhs=qT[g][hs, 512 * qc + c0:512 * (qc + 1)],
                                        start=True, stop=False,
                                        tile_position=(64 * h, 0),
                                    )
                                    nc.tensor.matmul(
                                        st[:, c0:],
                                        lhsT=onesB[p_h:p_h + 2, 0:128],
                                        rhs=ntau_row[p_h:p_h + 2,
                                                     512 * qc + c0:512 * (qc + 1)],
                                        start=False, stop=not (d >= 0),
                                        tile_position=(p_h, 0),
                                    )
                                    if d >= 0:
                                        if d == 3:
                                            nc.tensor.matmul(
                                                st[:, 256:384],
                                                lhsT=identB, rhs=fullB,
                                                start=False, stop=False,
                                            )
                                        nc.tensor.matmul(
                                            st[:, 128 * d:128 * (d + 1)],
                                            lhsT=identB, rhs=triLB,
                                            start=False, stop=True,
                                        )
                                if d < 0:
                                    # full-width block: one paired relu evict
                                    if g == 1 and kt % 2 == 0:
                                        nc.vector.tensor_scalar_max(a2, st2, 0.0)
                                    else:
                                        nc.scalar.activation(a2, st2, ACTF.Relu)
                                else:
                                    for h in range(2):
                                        hb = 512 * h
                                        if g == 1 and (kt + h) % 2 == 0:
                                            nc.vector.tensor_scalar_max(
                                                a2[:, hb + c0:hb + 512],
                                                st2[:, hb + c0:hb + 512], 0.0)
                                        else:
                                            nc.scalar.activation(
                                                a2[:, hb + c0:hb + 512],
                                                st2[:, hb + c0:hb + 512], ACTF.Relu)
                                for h in range(2):
                                    off = 64 * (2 * g + h)
                                    nc.tensor.matmul(
                                        pv[64 * h:64 * (h + 1), c0:],
                                        lhsT=vn[:, kt, off:off + 64],
                                        rhs=a2[:, 512 * h + c0:512 * (h + 1)],
                                        start=False, stop=False,
                                        tile_position=(0, 64 * h),
                                    )
                            nc.tensor.matmul(
                                pv, lhsT=zrow, rhs=ntau_row[0:1, 0:512],
                                start=False, stop=True,
                            )
                            nc.scalar.copy(opT[g][:, 512 * qc:512 * (qc + 1)], pv)

                            # ---- W_o projection (pipelined per q-chunk) ----
                            if g == 1:
                                for j in range(4 * qc, 4 * qc + 4):
                                    for ec in range(2):
                                        yp = zps.tile([128, 512], fp32, tag="z")
                                        for gg in range(2):
                                            nc.tensor.matmul(
                                                yp,
                                                lhsT=opT[gg][:, 128 * j:128 * (j + 1)],
                                                rhs=woT2[:, gg, 512 * ec:512 * (ec + 1)],
                                                start=(gg == 0), stop=(gg == 1),
                                            )
                                        ys = yout.tile([128, 512], fp32, tag="ys")
                                        if (2 * j + ec) % 2 == 0:
                                            nc.scalar.copy(ys, yp)
                                        else:
                                            nc.vector.tensor_copy(out=ys, in_=yp)
                                        nc.sync.dma_start(
                                            out=y_d[128 * j:128 * (j + 1),
                                                    512 * ec:512 * (ec + 1)],
                                            in_=ys)

                    proj_qk(0)
                    c16s0 = stage_a(0)       # DVE backbone starts here
                    solver_tau(0, c16s0)
                    proj_qk(1)               # PE fills under the g0 scan
                    proj_v()
                    stage_b(0)               # ACT window; DVE pulls A(g1)
                    c16s1 = stage_a(1)
                    solver_tau(1, c16s1)
                    stage_b(1)               # relus split ACT/DVE; W_o inline

    with tile.TileContext(nc) as tc:
        if niter > 1:
            with tc.For_i(0, niter):
                body(tc)
        else:
            body(tc)

    nc.compile()
    return nc


def host_prep(x, Wq, Wk, Wv, Wo, Lk=L):
    """Build the 8 per-core input dicts."""
    import ml_dtypes
    bf = ml_dtypes.bfloat16
    s = np.float32(1.0 / np.sqrt(HD))
    r = np.arange(128)
    triUA = np.where(r[None, :] > r[:, None], np.float32(NEG), 0.0).astype(bf)
    triLB = np.where(r[:, None] > r[None, :], np.float32(NEG), 0.0).astype(bf)
    fullB = np.full((128, 128), NEG, np.float32).astype(bf)
    identF = np.eye(128, dtype=np.float32)
    identB = np.eye(128, dtype=np.float32).astype(bf)
    nrinv = np.broadcast_to(
        (-1.0 / np.arange(1, NCAND + 1, dtype=np.float32))[None, :], (128, NCAND)
    ).copy()
    in_maps = []
    for c in range(N_CORES):
        b = c // 4
        h0 = HEADS_PER_CORE * (c % 4)
        rows = slice(HD * h0, HD * (h0 + HEADS_PER_CORE))  # 256 rows of W
        in_maps.append({
            "xT": np.ascontiguousarray(x[b, :Lk, :].T),                 # [D, Lk]
            "wqT": np.ascontiguousarray((Wq[rows, :] * s).T),           # [D, 256]
            "wkT": np.ascontiguousarray(Wk[rows, :].T),
            "wvT": np.ascontiguousarray(Wv[rows, :].T),
            "woT": np.ascontiguousarray(Wo[:, rows].T),                 # [256, D]
            "identF": identF, "identB": identB, "triUA": triUA,
            "triLB": triLB, "fullB": fullB,
            "ones": np.ones((128, 128), np.float32).astype(bf),
            "nrinv": nrinv,
        })
    return in_maps


_CACHED_NC = None


def kernel(x, Wq, Wk, Wv, Wo):
    global _CACHED_NC
    from concourse import bass_utils

    x = np.asarray(x, np.float32)
    in_maps = host_prep(x, np.asarray(Wq, np.float32), np.asarray(Wk, np.float32),
                        np.asarray(Wv, np.float32), np.asarray(Wo, np.float32))
    if _CACHED_NC is None:
        _CACHED_NC = build_program(L)
    res = bass_utils.run_bass_kernel_spmd(_CACHED_NC, in_maps, core_ids=list(range(N_CORES)))
    y = np.zeros((B, L, D), np.float32)
    for c in range(N_CORES):
        y[c // 4] += res.results[c]["y"]
    return y


if __name__ == "__main__":
    import reference
    inputs = {k: np.array(v) for k, v in reference.setup_inputs().items()}
    y = kernel(**inputs)
    print("kernel output:", y.shape, y.dtype, np.abs(y).max())
